# revision 39
# baseline (speedup 1.0000x reference)
"""Multi-head attention (B=1, n=4096, d=768, H=12) on 8 Trainium2 NeuronCores.

Sharding: 2 head-groups (6 heads = 384 dims) x 4 query-quarters (1024 q).
Core c = (hg = c // 4, sq = c % 4).

Per core:
  kT = Wk_hg @ K^T            [384, 4096]  (bf16, head-dim on partitions)
  qT = Wq_hg @ Q_sq^T         [384, 1024]
  v  = V @ Wv_hg^T (+ ones)   [4096, 6*65] (natural layout, 65th col = 1.0)
  S^T = k_h q_h^T  (pairs of heads row-packed on the PE, contraction dk=64)
  expS = exp(S^T / 8)   (ScalarE, PSUM->SBUF bf16, [128, 1024] per instr)
  o'^T_h[65, q] = [v_h | 1]^T @ expS    (row 64 = softmax denominators)
  oT_h = o'_h / sums  (DVE mul; reciprocal broadcast across partitions
                       via a K=1 f32r ones-matmul on the PE)
  Y_part = oT^T @ WoT_hg      [1024, 768] fp32

Host: Y[sq] = part(hg=0, sq) + part(hg=1, sq) + bo.

Scheduling: six (pair, q-chunk) attention phases, each ~33us of ScalarE exp
work; k/q projection chunks, deferred softmax normalization, and the output
projection are spread into the phases as just-in-time PE filler so the
ScalarE exp stream stays the critical path. The v projection rides the
first phase (attnV at key-tile kt consumes v[kt] just after it is made).
PSUM budget (8 banks): one shared triple-buffered pool (3x2 banks) serves
score tensors, projection chunks, the v projection and the reciprocal
broadcast; the other 2 banks hold the two attnV accumulators.
"""

import numpy as np
import ml_dtypes

import concourse.bass as bass  # noqa: F401  (bass types used via tile/bacc)
import concourse.mybir as mybir
import concourse.tile as tile
from concourse import bacc
from concourse.bass_utils import run_bass_kernel_spmd

P = 128
D = 768
NPOS = 4096
NQ = 1024          # queries per core
KD = D // P        # 6 contraction tiles for projections
MC = 3             # 384 head-dims per group = 3 chunks of 128 (2 heads each)
NKT = NPOS // P    # 32 key-position tiles
DK = 64
VW = 65            # v columns per head incl. ones column
QCH = 512          # query chunk (one PSUM bank)
BF16 = mybir.dt.bfloat16
F32 = mybir.dt.float32
FP = mybir.ActivationFunctionType

_CACHED_NC = None
LAST_RESULTS = None  # BassKernelResults from the most recent run (for test.py)


def build_program():
    nc = bacc.Bacc("TRN2", target_bir_lowering=False, debug=False)

    KT = nc.dram_tensor("KT", [D, NPOS], BF16, kind="ExternalInput")
    VTb = nc.dram_tensor("VTb", [NKT, P, D], BF16, kind="ExternalInput")
    QT = nc.dram_tensor("QT", [D, NQ], BF16, kind="ExternalInput")
    WqT = nc.dram_tensor("WqT", [D, MC * P], BF16, kind="ExternalInput")
    WkT = nc.dram_tensor("WkT", [D, MC * P], BF16, kind="ExternalInput")
    WvT = nc.dram_tensor("WvT", [D, MC * P], BF16, kind="ExternalInput")
    WoT = nc.dram_tensor("WoT", [MC * P, D], BF16, kind="ExternalInput")
    bq = nc.dram_tensor("bq", [P, MC], F32, kind="ExternalInput")
    bk = nc.dram_tensor("bk", [P, MC], F32, kind="ExternalInput")
    bvr = nc.dram_tensor("bvr", [P, MC * P], F32, kind="ExternalInput")
    Y = nc.dram_tensor("Y", [NQ, D], F32, kind="ExternalOutput")

    F32R = mybir.dt.float32r
    with tile.TileContext(nc) as tc:
        with (
            tc.tile_pool(name="const", bufs=1) as const,
            tc.tile_pool(name="persist", bufs=1) as persist,
            tc.tile_pool(name="vin", bufs=6) as vin,
            tc.tile_pool(name="expp", bufs=4) as expp,
            tc.tile_pool(name="small", bufs=3) as small,
            tc.tile_pool(name="ps_s", bufs=3, space="PSUM") as ps_s,
            tc.tile_pool(name="ps_o", bufs=2, space="PSUM") as ps_o,
        ):
            # ---- constants ----
            wq_sb = const.tile([P, KD, MC * P], BF16)
            nc.sync.dma_start(wq_sb, WqT.rearrange("(k p) m -> p k m", p=P))
            wk_sb = const.tile([P, KD, MC * P], BF16)
            nc.sync.dma_start(wk_sb, WkT.rearrange("(k p) m -> p k m", p=P))
            wv_sb = const.tile([P, KD, MC * P], BF16)
            nc.sync.dma_start(wv_sb, WvT.rearrange("(k p) m -> p k m", p=P))
            wo_sb = const.tile([P, MC, D], BF16)
            nc.sync.dma_start(wo_sb, WoT.rearrange("(k p) m -> p k m", p=P))
            bq_sb = const.tile([P, MC], F32)
            nc.sync.dma_start(bq_sb, bq[:, :])
            bk_sb = const.tile([P, MC], F32)
            nc.sync.dma_start(bk_sb, bk[:, :])
            bvr_sb = const.tile([P, MC * P], F32)
            nc.sync.dma_start(bvr_sb, bvr[:, :])
            ones_f32 = const.tile([1, DK], F32)
            nc.vector.memset(ones_f32, 1.0)
            ones_sb = const.tile([1, DK], F32R)
            with nc.allow_low_precision(reason="f32r ones for broadcast matmul"):
                nc.vector.tensor_copy(ones_sb, ones_f32)
            # trigger the exp table load while input DMAs stream
            warm_sb = const.tile([1, DK], F32)
            nc.scalar.activation(warm_sb, ones_f32, FP.Exp)

            # ---- persistent activations ----
            KT_res = persist.tile([P, KD, NPOS], BF16)
            QT_res = persist.tile([P, KD, NQ], BF16)
            KT_r = KT.rearrange("(k p) (t n) -> p k t n", p=P, n=QCH)
            for nt in range(NPOS // QCH):
                nc.sync.dma_start(KT_res[:, :, nt * QCH:(nt + 1) * QCH], KT_r[:, :, nt])
            QT_r = QT.rearrange("(k p) (t n) -> p k t n", p=P, n=QCH)
            for nt in range(NQ // QCH):
                nc.sync.dma_start(QT_res[:, :, nt * QCH:(nt + 1) * QCH], QT_r[:, :, nt])

            kT_sb = persist.tile([P, MC, NPOS], BF16)
            qT_sb = persist.tile([P, MC, NQ], BF16)
            v_sb = persist.tile([P, NKT, 6 * VW], BF16)
            oT_sb = persist.tile([P, MC, NQ], BF16)

            # ones columns of v (65th col per head)
            v_heads = v_sb.rearrange("p m (h x) -> p m h x", x=VW)
            nc.vector.memset(v_heads[:, :, :, DK], 1.0)

            def proj_chunk(mc, w_sb, b_sb, dst, n_total, nt):
                src = KT_res if n_total == NPOS else QT_res
                ps = ps_s.tile([P, QCH], F32, tag="s2")
                for kt in range(KD):
                    nc.tensor.matmul(
                        ps, w_sb[:, kt, mc * P:(mc + 1) * P],
                        src[:, kt, nt * QCH:(nt + 1) * QCH],
                        start=(kt == 0), stop=(kt == KD - 1),
                    )
                nc.vector.tensor_scalar_add(
                    dst[:, mc, nt * QCH:(nt + 1) * QCH], ps, b_sb[:, mc:mc + 1],
                )

            def v_proj(mt):
                vt = vin.tile([P, D], BF16, tag="vin")
                nc.sync.dma_start(vt, VTb[mt])
                ps = ps_s.tile([P, QCH], F32, tag="s2")
                for kt in range(KD):
                    nc.tensor.matmul(
                        ps[:, 0:MC * P], vt[:, kt * P:(kt + 1) * P],
                        wv_sb[:, kt, :],
                        start=(kt == 0), stop=(kt == KD - 1),
                    )
                nc.vector.tensor_tensor(
                    v_heads[:, mt, :, 0:DK],
                    ps[:, 0:MC * P].rearrange("p (h x) -> p h x", x=DK),
                    bvr_sb.rearrange("p (h x) -> p h x", x=DK),
                    mybir.AluOpType.add,
                )

            def scores(mc, qs, kt):
                s2 = ps_s.tile([P, 2, QCH], F32, tag="s2")
                nc.tensor.matmul(
                    s2[:, 0, :], kT_sb[0:DK, mc, kt * P:(kt + 1) * P],
                    qT_sb[0:DK, mc, qs], start=True, stop=True,
                )
                nc.tensor.matmul(
                    s2[:, 1, :], kT_sb[DK:P, mc, kt * P:(kt + 1) * P],
                    qT_sb[DK:P, mc, qs], start=True, stop=True,
                )
                return s2

            def out_proj(qt):
                y_sb = small.tile([P, D], F32, tag="y")
                for n0, nsz in ((0, 512), (512, 256)):
                    ps = ps_s.tile([P, QCH], F32, tag="s2")
                    for mc in range(MC):
                        nc.tensor.matmul(
                            ps[:, 0:nsz], oT_sb[:, mc, qt * P:(qt + 1) * P],
                            wo_sb[:, mc, n0:n0 + nsz],
                            start=(mc == 0), stop=(mc == MC - 1),
                        )
                    nc.vector.tensor_copy(y_sb[:, n0:n0 + nsz], ps[:, 0:nsz])
                nc.sync.dma_start(Y[qt * P:(qt + 1) * P, :], y_sb)

            def normalize(mc, qc, oc_pair):
                """Deferred: divide o' by the softmax sums, write oT_sb."""
                qs = slice(qc * QCH, (qc + 1) * QCH)
                for idx, oc in ((0, oc_pair[0]), (1, oc_pair[1])):
                    r = small.tile([1, QCH], F32R, tag="r")
                    with nc.allow_low_precision(reason="f32r reciprocal for broadcast"):
                        nc.vector.reciprocal(r, oc[DK:DK + 1, :])
                    rr_ps = ps_s.tile([DK, QCH], F32, tag="s2")
                    nc.tensor.matmul(
                        rr_ps, ones_sb, r, start=True, stop=True,
                    )
                    rr = small.tile([DK, QCH], F32, tag="rr")
                    nc.vector.tensor_copy(rr, rr_ps)
                    if idx == 0:
                        nc.vector.tensor_tensor(
                            oT_sb[0:DK, mc, qs], oc[0:DK, :], rr,
                            mybir.AluOpType.mult,
                        )
                    else:
                        ob = small.tile([DK, QCH], BF16, tag="ob")
                        nc.vector.tensor_tensor(
                            ob, oc[0:DK, :], rr, mybir.AluOpType.mult,
                        )
                        nc.sync.dma_start(oT_sb[DK:P, mc, qs], ob)

            def attn(mc, qc, jit_work=None, defer_norm=True):
                """Attention for head pair mc over query chunk qc.

                jit_work: {kt: [callbacks]} — PE work emitted at exactly
                iteration kt (just-in-time v/kT chunks, spread filler).
                defer_norm: copy o' to SBUF and return a normalize closure
                for a later phase; if False, normalize straight from PSUM
                (shorter chain — used for the final phase).
                """
                qs = slice(qc * QCH, (qc + 1) * QCH)
                jit = jit_work or {}
                assert all(0 <= kt < NKT for kt in jit), sorted(jit)
                oA = ps_o.tile([VW, QCH], F32, tag="o")
                oB = ps_o.tile([VW, QCH], F32, tag="o")
                s2_cur = scores(mc, qs, 0)
                for kt in range(NKT):
                    e = expp.tile([P, 2, QCH], BF16, tag="e")
                    nc.scalar.activation(e, s2_cur, FP.Exp, scale=0.125)
                    if kt + 1 < NKT:
                        s2_next = scores(mc, qs, kt + 1)
                    for cb in jit.get(kt, ()):
                        cb()
                    nc.tensor.matmul(
                        oA, v_sb[:, kt, (2 * mc) * VW:(2 * mc) * VW + VW],
                        e[:, 0, :],
                        start=(kt == 0), stop=(kt == NKT - 1),
                    )
                    nc.tensor.matmul(
                        oB, v_sb[:, kt, (2 * mc + 1) * VW:(2 * mc + 1) * VW + VW],
                        e[:, 1, :],
                        start=(kt == 0), stop=(kt == NKT - 1),
                    )
                    if kt + 1 < NKT:
                        s2_cur = s2_next
                if not defer_norm:
                    normalize(mc, qc, (oA, oB))
                    return None
                # free the o-psum banks quickly: copy to SBUF, normalize later
                ocs = []
                for o in (oA, oB):
                    oc = small.tile([VW, QCH], F32, tag="oc")
                    nc.vector.tensor_copy(oc, o)
                    ocs.append(oc)
                return lambda: normalize(mc, qc, ocs)

            # ---- emission order: overlap PE proj phases with ACT-bound attn ----
            def kchunk(mc, nt):
                return lambda: proj_chunk(mc, wk_sb, bk_sb, kT_sb, NPOS, nt)

            def qchunk(mc, nt):
                return lambda: proj_chunk(mc, wq_sb, bq_sb, qT_sb, NQ, nt)

            def add_jit(jit, kt, cb):
                jit.setdefault(kt, []).append(cb)

            # startup: all of kT0/qT0 (uncontended PSUM slots), then
            # phase 0 = attn(0,0) with the v-projection JIT one step ahead.
            for n in range(NPOS // QCH):
                kchunk(0, n)()
            for n in range(NQ // QCH):
                qchunk(0, n)()
            jit0 = {}
            v_proj(0)
            for kt in range(NKT):
                if kt + 1 < NKT:
                    add_jit(jit0, kt, (lambda m=kt + 1: v_proj(m)))
            n00 = attn(0, 0, jit0)

            # phase 1 = attn(0,1): kT1 chunks 0..3, qT1 chunk 0, normalize(0,0)
            jit1 = {}
            for i, cb in enumerate([kchunk(1, n) for n in range(4)]
                                   + [qchunk(1, 0), n00]):
                add_jit(jit1, 2 * i + 1, cb)
            n01 = attn(0, 1, jit1)

            # phase 2 = attn(1,0): kT1 chunks 4..7 JIT, qT1 chunk 1, norm(0,1)
            jit2 = {}
            for n in range(4, NPOS // QCH):
                add_jit(jit2, 4 * n - 15, kchunk(1, n))
            add_jit(jit2, 19, qchunk(1, 1))
            add_jit(jit2, 25, n01)
            n10 = attn(1, 0, jit2)

            # phase 3 = attn(1,1): kT2 chunks 0..3, qT2 chunk 0, norm(1,0)
            jit3 = {}
            for i, cb in enumerate([kchunk(2, n) for n in range(4)]
                                   + [qchunk(2, 0), n10]):
                add_jit(jit3, 2 * i + 1, cb)
            n11 = attn(1, 1, jit3)

            # phase 4 = attn(2,0): kT2 chunks 4..7 JIT, qT2 chunk 1, norm(1,1)
            jit4 = {}
            for n in range(4, NPOS // QCH):
                add_jit(jit4, 4 * n - 15, kchunk(2, n))
            add_jit(jit4, 19, qchunk(2, 1))
            add_jit(jit4, 25, n11)
            n20 = attn(2, 0, jit4)

            # phase 5 = attn(2,1): norm(2,0) + out-projection of query half 0
            jit5 = {}
            add_jit(jit5, 1, n20)
            for qt in range(4):
                add_jit(jit5, 3 + 4 * qt, (lambda q=qt: out_proj(q)))
            attn(2, 1, jit5, defer_norm=False)

            for qt in range(4, NQ // P):
                out_proj(qt)

    nc.compile()
    return nc


def kernel(**inputs):
    global _CACHED_NC, LAST_RESULTS
    bf = ml_dtypes.bfloat16
    f32 = np.float32

    Q = np.asarray(inputs["Q"], dtype=f32)
    K = np.asarray(inputs["K"], dtype=f32)
    V = np.asarray(inputs["V"], dtype=f32)
    Wq = np.asarray(inputs["Wq"], dtype=f32)
    bq = np.asarray(inputs["bq"], dtype=f32)
    Wk = np.asarray(inputs["Wk"], dtype=f32)
    bk = np.asarray(inputs["bk"], dtype=f32)
    Wv = np.asarray(inputs["Wv"], dtype=f32)
    bv = np.asarray(inputs["bv"], dtype=f32)
    Wo = np.asarray(inputs["Wo"], dtype=f32)
    bo = np.asarray(inputs["bo"], dtype=f32)

    KTh = np.ascontiguousarray(K[0].T).astype(bf)                 # [768, 4096]
    VT = V[0].T                                                   # [768, 4096]
    VTb = np.ascontiguousarray(
        VT.reshape(KD, P, NKT, P).transpose(2, 1, 0, 3).reshape(NKT, P, D)
    ).astype(bf)
    QTs = [
        np.ascontiguousarray(Q[0, sq * NQ:(sq + 1) * NQ, :].T).astype(bf)
        for sq in range(4)
    ]

    per_hg = []
    for hg in range(2):
        sl = slice(hg * 384, (hg + 1) * 384)
        per_hg.append(dict(
            WqT=np.ascontiguousarray(Wq[sl, :].T).astype(bf),
            WkT=np.ascontiguousarray(Wk[sl, :].T).astype(bf),
            WvT=np.ascontiguousarray(Wv[sl, :].T).astype(bf),
            WoT=np.ascontiguousarray(Wo[:, sl].T).astype(bf),
            bq=np.ascontiguousarray(bq[sl].reshape(MC, P).T).astype(f32),
            bk=np.ascontiguousarray(bk[sl].reshape(MC, P).T).astype(f32),
            bvr=np.ascontiguousarray(
                np.broadcast_to(bv[sl][None, :], (P, 384))
            ).astype(f32),
        ))

    in_maps = []
    for c in range(8):
        hg, sq = c // 4, c % 4
        in_maps.append(dict(
            KT=KTh, VTb=VTb, QT=QTs[sq], **per_hg[hg],
        ))

    if _CACHED_NC is None:
        _CACHED_NC = build_program()
    nc = _CACHED_NC

    LAST_RESULTS = run_bass_kernel_spmd(nc, in_maps, core_ids=list(range(8)))
    parts = [r["Y"] for r in LAST_RESULTS.results]

    out = np.empty((1, NPOS, D), dtype=f32)
    for sq in range(4):
        out[0, sq * NQ:(sq + 1) * NQ] = parts[sq] + parts[4 + sq] + bo[None, :]
    return out
